# revision 7
# baseline (speedup 1.0000x reference)
"""CoevolExtractor fused kernel for 8x trn2 NeuronCores (Bass/Tile).

Computation (reference):
    pair[b,i,l,j,m] = sum_n x_down[b,n,i,j] * x_down_w[b,n,l,m]
    pair = LayerNorm_{(j,m)}(pair) * a_2 + b_2        (eps=1e-5, biased var)
    out  = pair @ W + b                               # (1, L, L, 128)

Strategy: shard i (first residue axis) across 8 cores (24 i's each).
Per core:
  pair slab = A^T @ B (fp32r matmuls, K=256) in layout [(i4,j) x (l,m)],
  row-tile outer so the Linear can interleave; PSUM->SBUF bf16 copies all
  on ACT (they pace phase A while PE fills with early Linear m-steps).
  LayerNorm folded into the projection:
     out[t,f] = invstd[t] * ( (pair @ W')[t,f] + s[f]*(-mean[t]) + bconst[f]*std[t] )
  mean via factorized row-sums (sA^T sB, bf16); sumsq via DVE bf16
  squares (TensorTensor, 2x mode) + gpsimd first fold + DVE fold tree
  over m + per-row-tile indicator matmuls over j.
  Linear in 12 (rp,g) units: 32 K=32 m-strided matmuls accumulate one
  PSUM bank, rank-1 correction appended, invstd column scale (gpsimd
  broadcast + DVE mul), DMA out in [f, t] layout. Units pipeline with a
  lag so only the last unit's epilogue is exposed.
"""

import os
from contextlib import ExitStack

import ml_dtypes
import numpy as np

import concourse.bass as bass
import concourse.tile as tile
from concourse import bacc, mybir
from concourse.bass_utils import run_bass_kernel_spmd

F32 = mybir.dt.float32
F32R = mybir.dt.float32r
BF16 = mybir.dt.bfloat16

B, N, L, J = 1, 256, 192, 32
D2 = J * J          # 1024
F = 128             # n_feat_out
NCORES = 8
LI = L // NCORES    # 24 i's per core
NK = N // 128       # 2 contraction k-tiles
NRT = LI * J // 128  # 6 row tiles of (i4, j)
CB = 512            # pair col-block width
NCB = L * J // CB   # 12 col blocks
NRP = NRT // 2      # 3 row-tile pairs
NU = 4 * NRP        # 12 linear units (rp, g)
HW = L * J // 2     # 3072 cols per half row-tile
EPS = 1e-5
AX = mybir.AxisListType
ALU = mybir.AluOpType
ACTF = mybir.ActivationFunctionType


def build_kernel(ctx: ExitStack, tc: tile.TileContext, xa, xb, wrep, sb2, bones, y):
    nc = tc.nc

    const = ctx.enter_context(tc.tile_pool(name="const", bufs=1))
    bpool = ctx.enter_context(tc.tile_pool(name="b2", bufs=1))
    prpool = ctx.enter_context(tc.tile_pool(name="pairsb", bufs=1))
    sqpool = ctx.enter_context(tc.tile_pool(name="sqp", bufs=2))
    spool = ctx.enter_context(tc.tile_pool(name="ssqp", bufs=3))
    work = ctx.enter_context(tc.tile_pool(name="work", bufs=1))
    ipool = ctx.enter_context(tc.tile_pool(name="ipool", bufs=4))
    opool = ctx.enter_context(tc.tile_pool(name="opool", bufs=3))
    bank = ctx.enter_context(tc.tile_pool(name="bank", bufs=7, space="PSUM"))
    statp = ctx.enter_context(tc.tile_pool(name="statp", bufs=1, space="PSUM"))

    # ---- input DMAs, consumption order; wrep head chunk early for fills ----
    a_t = []
    for k in range(NK):
        at = const.tile([128, LI * J], F32R, tag=f"a{k}")
        nc.sync.dma_start(at[:], xa[k * 128:(k + 1) * 128, :])
        a_t.append(at)
    wrep_t = const.tile([128, J * F], BF16, tag="wrep")
    b_t = [[None] * NCB for _ in range(NK)]
    for cb in range(NCB):
        for k in range(NK):
            bt = bpool.tile([128, CB], F32R, tag=f"b{k}_{cb}")
            nc.sync.dma_start(bt[:], xb[k * 128:(k + 1) * 128, cb * CB:(cb + 1) * CB])
            b_t[k][cb] = bt
        if cb == 2:
            # first 8 m-slices of the Linear weights (for early fills)
            nc.sync.dma_start(wrep_t[:, 0:8 * F], wrep[:, 0:8 * F])
    nc.sync.dma_start(wrep_t[:, 8 * F:J * F], wrep[:, 8 * F:J * F])
    sb2_t = const.tile([128, F], BF16, tag="sb2")
    nc.sync.dma_start(sb2_t[:], sb2[:])
    bones_t = const.tile([128, NRT * LI], BF16, tag="bones")
    nc.sync.dma_start(bones_t[:], bones[:])

    pair_sb = [prpool.tile([128, 2 * L * J], BF16, tag=f"prp{rp}", name=f"prp{rp}")
               for rp in range(NRP)]

    eps24 = work.tile([LI, 1], F32, tag="eps24")
    nc.gpsimd.memset(eps24[:], EPS)
    # staged per-t rows, replicated at partitions {32g, 32g+1}, zero-padded K=32
    stage2 = work.tile([128, LI * L], BF16, tag="stage2")
    nc.gpsimd.memset(stage2[:], 0.0)
    stage_inv = work.tile([1, LI * L], F32, tag="stage_inv")

    statb = statp.tile([LI, 2 * L], F32, tag="statb")
    ssq_ps = statb[:, 0:L]
    mean_ps = statb[:, L:2 * L]

    # ---- factorized mean inputs: sa = sum_j A, sb = sum_m B (bf16) ----
    sa_t = [work.tile([128, LI], BF16, tag=f"sa{k}", name=f"sa{k}")
            for k in range(NK)]
    sb_t = [work.tile([128, L], BF16, tag=f"sb{k}", name=f"sb{k}")
            for k in range(NK)]
    _sbq = [(k, cb) for cb in range(NCB) for k in range(NK)]

    def emit_sb_reduce(n):
        with nc.allow_low_precision(reason="bf16 row-sums; product sum in psum f32"):
            for _ in range(n):
                if not _sbq:
                    return
                k, cb = _sbq.pop(0)
                nc.vector.tensor_reduce(
                    sb_t[k][:, cb * 16:(cb + 1) * 16],
                    b_t[k][cb][:].rearrange("p (l m) -> p l m", m=J),
                    axis=AX.X, op=ALU.add)

    def emit_sa_reduce():
        with nc.allow_low_precision(reason="bf16 row-sums"):
            for k in range(NK):
                nc.vector.tensor_reduce(
                    sa_t[k][:], a_t[k][:].rearrange("p (i j) -> p i j", j=J),
                    axis=AX.X, op=ALU.add)

    # ---- Linear m-loop fill machinery (units u = 4*rp + g) ----
    psl_u = {}
    fill_state = {"u": 0, "m": 0}

    def emit_msteps(nsteps):
        while nsteps > 0 and fill_state["u"] < NU:
            u, m = fill_state["u"], fill_state["m"]
            rp, g = u // 4, u % 4
            if m == 0:
                psl_u[u] = bank.tile([128, CB], F32, tag="bank",
                                     name=f"psl{u}")[:, 0:2 * L]
            prp4 = pair_sb[rp][:].rearrange("p (r l m) -> p r l m", r=2, m=J)
            take = min(nsteps, J - m)
            for mm in range(m, m + take):
                nc.tensor.matmul(
                    psl_u[u],
                    wrep_t[32 * g:32 * (g + 1), mm * F:(mm + 1) * F],
                    prp4[32 * g:32 * (g + 1), :, :, mm],
                    start=(mm == 0), stop=False,
                    tile_position=(32 * g, 0),
                    skip_group_check=True)
            nsteps -= take
            if m + take == J:
                fill_state["u"] += 1
                fill_state["m"] = 0
            else:
                fill_state["m"] = m + take

    # ---- per-half-rt stats: square (DVE 2x) + gpsimd fold1 + DVE folds ----
    ssq_t = [None] * NRT

    def emit_half_stats(rt, h):
        pslice = pair_sb[rt // 2][:, (rt % 2) * L * J + h * HW:
                                  (rt % 2) * L * J + (h + 1) * HW]
        sq = sqpool.tile([128, HW], BF16, tag="sq")
        with nc.allow_low_precision(reason="bf16 squares; j-sum in psum f32"):
            nc.vector.tensor_mul(sq[:], pslice, pslice)
            sqv = sq[:].rearrange("p (l m) -> p l m", m=J)
            nc.gpsimd.tensor_add(sqv[:, :, 0:16], sqv[:, :, 0:16], sqv[:, :, 16:32])
            half = 8
            while half >= 2:
                nc.vector.tensor_add(
                    sqv[:, :, 0:half], sqv[:, :, 0:half], sqv[:, :, half:2 * half])
                half //= 2
            if h == 0:
                ssq_t[rt] = spool.tile([128, L], BF16, tag="ssq", name=f"ssq{rt}")
            nc.vector.tensor_add(
                ssq_t[rt][:, h * 96:(h + 1) * 96], sqv[:, :, 0], sqv[:, :, 1])

    def emit_ssq_mm(rt):
        nc.tensor.matmul(ssq_ps, bones_t[:, rt * LI:(rt + 1) * LI], ssq_t[rt][:],
                         start=(rt == 0), stop=(rt == NRT - 1),
                         skip_group_check=True)

    # ---- phase A: pair matmuls + ACT copies + stats, fills from rt2 ----
    for rt in range(NRT):
        rp, rt2 = rt // 2, rt % 2
        for cb in range(NCB):
            pp = bank.tile([128, CB], F32, tag="bank")
            for k in range(NK):
                nc.tensor.matmul(
                    pp[:],
                    a_t[k][:, rt * 128:(rt + 1) * 128],
                    b_t[k][cb][:],
                    start=(k == 0),
                    stop=(k == NK - 1),
                )
            nc.scalar.activation(
                pair_sb[rp][:, rt2 * L * J + cb * CB: rt2 * L * J + (cb + 1) * CB],
                pp[:], ACTF.Copy)
            if rt >= 2:
                emit_msteps(2)
            if cb == 5 or cb == 11:
                emit_half_stats(rt, cb // 6)
                emit_sb_reduce(2)
        if rt == 0:
            emit_sa_reduce()
        if rt >= 2:
            emit_ssq_mm(rt - 2)

    emit_sb_reduce(26)

    # ---- stats finalize ----
    def emit_finalize():
        for rt in range(NRT - 2, NRT):
            emit_ssq_mm(rt)
        for k in range(NK):
            nc.tensor.matmul(mean_ps, sa_t[k][:], sb_t[k][:],
                             start=(k == 0), stop=(k == NK - 1),
                             skip_group_check=True)
        mean24 = work.tile([LI, L], F32, tag="mean24")
        nc.vector.tensor_scalar_mul(mean24[:], mean_ps, 1.0 / D2)
        mean2 = work.tile([LI, L], F32, tag="mean2")
        nc.vector.tensor_mul(mean2[:], mean24[:], mean24[:])
        var24 = work.tile([LI, L], F32, tag="var24")
        nc.vector.scalar_tensor_tensor(
            var24[:], ssq_ps, 1.0 / D2, mean2[:], op0=ALU.mult, op1=ALU.subtract)
        std24 = work.tile([LI, L], F32, tag="std24")
        nc.scalar.activation(std24[:], var24[:], ACTF.Sqrt, bias=eps24[:])
        invstd24 = work.tile([LI, L], F32, tag="invstd24")
        nc.vector.reciprocal(invstd24[:], std24[:])
        mneg24 = work.tile([LI, L], BF16, tag="mneg24")
        nc.vector.tensor_scalar_mul(mneg24[:], mean24[:], -1.0)
        stdbf24 = work.tile([LI, L], BF16, tag="stdbf24")
        nc.vector.tensor_copy(stdbf24[:], std24[:])
        for g in range(4):
            nc.sync.dma_start(
                stage2[32 * g:32 * g + 1, 0:LI * L].rearrange("o (i l) -> o i l", i=LI),
                mneg24[:])
            nc.sync.dma_start(
                stage2[32 * g + 1:32 * g + 2, 0:LI * L].rearrange("o (i l) -> o i l", i=LI),
                stdbf24[:])
        nc.sync.dma_start(stage_inv[0:1, :].rearrange("o (i l) -> o i l", i=LI),
                          invstd24[:])

    st4 = stage2[:].rearrange("p (h g l) -> p h g l", g=4, l=L)

    def emit_tail(u):
        rp, g = u // 4, u % 4
        psl = psl_u[u]
        # rank-1 corrections: s x (-mean) + bconst x std
        nc.tensor.matmul(
            psl,
            sb2_t[32 * g:32 * (g + 1), :],
            st4[32 * g:32 * (g + 1), 2 * rp:2 * rp + 2, g, :],
            start=False, stop=True, tile_position=(32 * g, 0),
            skip_group_check=True)
        # invstd broadcast (gpsimd) + column scale (DVE), write out [f, t]
        inv_bc = ipool.tile([128, 2 * L], F32, tag="inv_bc")
        for rt2 in range(2):
            i = (2 * rp + rt2) * 4 + g
            nc.gpsimd.partition_broadcast(
                inv_bc[:, rt2 * L:(rt2 + 1) * L],
                stage_inv[0:1, i * L:(i + 1) * L])
        out_sb = opool.tile([128, 2 * L], F32, tag="out_sb")
        nc.vector.tensor_mul(out_sb[:], psl, inv_bc[:])
        y4 = y[:, :].rearrange("f (h g l) -> f h g l", g=4, l=L)
        nc.sync.dma_start(
            y4[:, 2 * rp:2 * rp + 2, g, :],
            out_sb[:].rearrange("f (h l) -> f h l", l=L))

    # ---- phase C: remaining units, finalize slotted in, tails lag LAG ----
    LAG = 5
    # finish any partially-filled unit, then one more before the stats tail
    if fill_state["m"] != 0:
        emit_msteps(J - fill_state["m"])
    if fill_state["u"] < NU:
        emit_msteps(J)
    emit_finalize()
    next_tail = 0
    while next_tail < NU:
        if fill_state["u"] < NU:
            emit_msteps(J)
        limit = NU if fill_state["u"] >= NU else max(0, fill_state["u"] - LAG)
        while next_tail < limit:
            emit_tail(next_tail)
            next_tail += 1


def build_program():
    nc = bacc.Bacc("TRN2", target_bir_lowering=False, debug=False,
                   num_devices=NCORES)
    xa = nc.dram_tensor("xa", [N, LI * J], F32R, kind="ExternalInput").ap()
    xb = nc.dram_tensor("xb", [N, L * J], F32R, kind="ExternalInput").ap()
    wrep = nc.dram_tensor("wrep", [128, J * F], BF16, kind="ExternalInput").ap()
    sb2 = nc.dram_tensor("sb2", [128, F], BF16, kind="ExternalInput").ap()
    bones = nc.dram_tensor("bones", [128, NRT * LI], BF16, kind="ExternalInput").ap()
    y = nc.dram_tensor("y", [F, LI * L], F32, kind="ExternalOutput").ap()

    reps = int(os.environ.get("COEVOL_REPS", "1"))
    with tile.TileContext(nc) as tc:
        for _ in range(reps):
            with ExitStack() as ctx:
                build_kernel(ctx, tc, xa, xb, wrep, sb2, bones, y)
    nc.compile()
    return nc


def host_inputs(x_down, x_down_w, a_2, b_2, W, b):
    """Host-side prep: reshapes + weight prepacking. Returns per-core input maps."""
    A2 = np.ascontiguousarray(x_down.reshape(N, L * J).astype(np.float32))
    B2 = np.ascontiguousarray(x_down_w.reshape(N, L * J).astype(np.float32))
    Wp = (a_2.astype(np.float64)[:, None] * W.astype(np.float64))
    s_row = Wp.sum(axis=0)
    bconst = b_2.astype(np.float64) @ W.astype(np.float64) + b.astype(np.float64)
    wrep = np.tile(Wp.reshape(J, J * F), (4, 1)).astype(ml_dtypes.bfloat16)
    sb2 = np.zeros((128, F), dtype=ml_dtypes.bfloat16)
    for g in range(4):
        sb2[32 * g] = s_row.astype(ml_dtypes.bfloat16)
        sb2[32 * g + 1] = bconst.astype(ml_dtypes.bfloat16)
    # per-row-tile j-reduction indicators: bones[:, rt*LI + i'] = 1 where the
    # partition belongs to group g and i' == 4*rt + g
    bones = np.zeros((128, NRT * LI), dtype=ml_dtypes.bfloat16)
    for rt in range(NRT):
        for g in range(4):
            bones[32 * g:32 * (g + 1), rt * LI + 4 * rt + g] = 1.0
    in_maps = []
    for c in range(NCORES):
        in_maps.append({
            "xa": np.ascontiguousarray(A2[:, c * LI * J:(c + 1) * LI * J]),
            "xb": B2,
            "wrep": wrep,
            "sb2": sb2,
            "bones": bones,
        })
    return in_maps


_NC_CACHE = {}


def _get_program():
    if "nc" not in _NC_CACHE:
        _NC_CACHE["nc"] = build_program()
    return _NC_CACHE["nc"]


def kernel(**inputs) -> np.ndarray:
    nc = _get_program()
    inputs = {k: np.asarray(v) for k, v in inputs.items()}
    in_maps = host_inputs(**inputs)
    trace = bool(int(os.environ.get("COEVOL_TRACE", "0")))
    res = run_bass_kernel_spmd(nc, in_maps, list(range(NCORES)), trace=trace)
    if trace:
        _NC_CACHE["last_result"] = res
    # per-core y is [F, LI*L]; unshard to (B, L, L, F)
    slabs = [res.results[c]["y"].reshape(F, LI, L).transpose(1, 2, 0)
             for c in range(NCORES)]
    return np.concatenate(slabs, axis=0).reshape(B, L, L, F)
